# revision 2
# baseline (speedup 1.0000x reference)
"""Trainium2 Bass kernel for DiffCompressModule.

Reference computation (B=4, S=512, D_IN=D_OUT=4096):
    out = h @ W.T + b + coeff[b] * (h @ (2*mask[b] - 1))

Fused form used here (one matmul instead of two):
    out[b] = h[b] @ M_b + bias,   M_b = W.T + coeff[b] * (2*mask[b] - 1)

The host re-encodes each input tensor (layout + dtype only, no cross-tensor
math): h and W.T are shipped pre-transposed in bf16, the {0,1} mask as
fp8e4 (both values exact). On-device, ACT decodes t = 2c*mask - c (reading
fp8, writing bf16), DVE adds W.T to form M_b, and the fused matmul runs in
bf16 with fp32 PSUM accumulation. All DMAs are plain HWDGE copies; DMA cost
is charged on SBUF-side bytes, so per core it moves ~28MB-equivalent
(mask 8 + h 8 + W 8 + out 4) while PE does 512 x [128x128]@[128x512]
bf16 matmuls (~109us) - the kernel is PE-bound.

Sharding over 8 cores: 4 out-feature groups x 2 batch groups.
Each core: h [2,4096,512] bf16, W [4096,1024] bf16, bias [1024] f32,
coeff [2] f32, mask [2,4096,1024] fp8 -> out [2,512,1024] f32.
"""

import numpy as np

import concourse.bass as bass
import concourse.mybir as mybir
from concourse import tile
from concourse.bass_utils import run_bass_kernel_spmd

B, S, D = 4, 512, 4096
O_FULL = 4096
N_CORES = 8
OG, BG = 4, 2  # out-feature groups x batch groups
O_SH = O_FULL // OG  # 1024 out features per core
B_SH = B // BG  # 2 batches per core
HALF = 512  # PSUM bank holds 512 fp32 per partition
KC = D // 128  # 32 contraction chunks
SC = S // 128  # 4 s chunks
# kc-chunk grouping per (DMA + decode) unit: fine-grained at the start so
# the first matmul is ready early, coarse after to amortize overheads.
UNITS = (1, 1, 1, 1, 4, 4, 4, 4, 4, 4, 4)
dt = mybir.dt

_CACHE = {}


def _split_sync_waits(nc, max_waits=1):
    # CoreV3 walrus rejects instructions with more than one semaphore wait
    # ("Too many sync wait commands"). Splitting the waits across preceding
    # same-engine NOPs is equivalent (the sequencer blocks on each in turn).
    ctr = 0
    for fn in nc.m.functions:
        for bb in fn.blocks:
            insts = bb.instructions
            if not any(
                i.sync_info is not None and len(i.sync_info.on_wait) > max_waits
                for i in insts
            ):
                continue
            new_list = []
            for ins in insts:
                si = ins.sync_info
                if si is not None and len(si.on_wait) > max_waits:
                    waits = list(si.on_wait)
                    head, tail = waits[:-max_waits], waits[-max_waits:]
                    for k in range(0, len(head), max_waits):
                        nop = mybir.InstNoOp(
                            name=f"waitsplit-{ctr}",
                            engine=ins.engine,
                            ins=[],
                            outs=[],
                            sync_info=mybir.SyncInfo(
                                on_wait=head[k : k + max_waits], on_update=[]
                            ),
                        )
                        ctr += 1
                        new_list.append(nop)
                    ins.sync_info = mybir.SyncInfo(
                        on_wait=tail, on_update=list(si.on_update)
                    )
                new_list.append(ins)
            bb.instructions = new_list


def _build_nc():
    nc = bass.Bass("TRN2", target_bir_lowering=False, debug=False)
    h = nc.dram_tensor("h", [B_SH, D, S], dt.bfloat16, kind="ExternalInput").ap()
    W = nc.dram_tensor("W", [D, O_SH], dt.bfloat16, kind="ExternalInput").ap()
    bias = nc.dram_tensor("bias", [O_SH], dt.float32, kind="ExternalInput").ap()
    coeff = nc.dram_tensor("coeff", [B_SH], dt.float32, kind="ExternalInput").ap()
    mask = nc.dram_tensor(
        "mask", [B_SH, D, O_SH], dt.float8e4, kind="ExternalInput"
    ).ap()
    out = nc.dram_tensor("out", [B_SH, S, O_SH], dt.float32, kind="ExternalOutput").ap()

    with tile.TileContext(nc) as tc:
        with (
            tc.tile_pool(name="const", bufs=1) as const_pool,
            tc.tile_pool(name="wt", bufs=len(UNITS)) as wt_pool,
            tc.tile_pool(name="ht", bufs=6) as ht_pool,
            tc.tile_pool(name="mk", bufs=4) as mk_pool,
            tc.tile_pool(name="tt", bufs=2) as t_pool,
            tc.tile_pool(name="m", bufs=3) as m_pool,
            tc.tile_pool(name="ost", bufs=4) as out_pool,
            tc.tile_pool(name="acc", bufs=8, space="PSUM") as acc_pool,
        ):
            coeff_bc = const_pool.tile([128, B_SH], dt.float32)
            nc.sync.dma_start(
                coeff_bc[:], bass.AP(coeff.tensor, 0, [[0, 128], [1, B_SH]])
            )
            c2 = const_pool.tile([128, B_SH], dt.float32)
            cneg = const_pool.tile([128, B_SH], dt.float32)
            nc.vector.tensor_scalar_mul(c2[:], coeff_bc[:], 2.0)
            nc.vector.tensor_scalar_mul(cneg[:], coeff_bc[:], -1.0)
            bias_bc = const_pool.tile([128, O_SH], dt.float32)
            nc.sync.dma_start(
                bias_bc[:], bass.AP(bias.tensor, 0, [[0, 128], [1, O_SH]])
            )

            wt = []  # per-unit W.T tiles [128, u*O_SH] bf16, resident

            for b in range(B_SH):
                accs = [
                    acc_pool.tile([128, HALF], dt.float32, tag="acc", name="acc")
                    for _ in range(SC * 2)
                ]
                kc = 0
                for ui, u in enumerate(UNITS):
                    kc0 = kc
                    if b == 0:
                        w_t = wt_pool.tile([128, u * O_SH], dt.bfloat16, name="wt")
                        nc.sync.dma_start(
                            w_t[:],
                            bass.AP(
                                W.tensor,
                                kc0 * 128 * O_SH,
                                [[O_SH, 128], [128 * O_SH, u], [1, O_SH]],
                            ),
                        )
                        wt.append(w_t)
                    mk = mk_pool.tile([128, u * O_SH], dt.float8e4, name="mk")
                    nc.sync.dma_start(
                        mk[:],
                        bass.AP(
                            mask.tensor,
                            (b * D + kc0 * 128) * O_SH,
                            [[O_SH, 128], [128 * O_SH, u], [1, O_SH]],
                        ),
                    )
                    ht = ht_pool.tile([128, u * S], dt.bfloat16, name="ht")
                    nc.sync.dma_start(
                        ht[:],
                        bass.AP(
                            h.tensor,
                            (b * D + kc0 * 128) * S,
                            [[S, 128], [128 * S, u], [1, S]],
                        ),
                    )
                    t_sb = t_pool.tile([128, u * O_SH], dt.bfloat16, name="tsb")
                    nc.scalar.activation(
                        t_sb[:],
                        mk[:],
                        mybir.ActivationFunctionType.Identity,
                        bias=cneg[:, b : b + 1],
                        scale=c2[:, b : b + 1],
                    )
                    m = m_pool.tile([128, u * O_SH], dt.bfloat16, name="m")
                    nc.vector.tensor_tensor(
                        m[:], t_sb[:], wt[ui][:], mybir.AluOpType.add
                    )
                    for j in range(u):
                        for sc in range(SC):
                            for half in range(2):
                                nc.tensor.matmul(
                                    accs[sc * 2 + half][:],
                                    ht[:, j * S + sc * 128 : j * S + (sc + 1) * 128],
                                    m[
                                        :,
                                        j * O_SH + half * HALF : j * O_SH
                                        + (half + 1) * HALF,
                                    ],
                                    start=(kc == 0),
                                    stop=(kc == KC - 1),
                                )
                        kc += 1
                for sc in range(SC):
                    for half in range(2):
                        o_sb = out_pool.tile([128, HALF], dt.float32, name="osb")
                        nc.vector.tensor_tensor(
                            o_sb[:],
                            accs[sc * 2 + half][:],
                            bias_bc[:, half * HALF : (half + 1) * HALF],
                            mybir.AluOpType.add,
                        )
                        nc.gpsimd.dma_start(
                            out[
                                b,
                                sc * 128 : (sc + 1) * 128,
                                half * HALF : (half + 1) * HALF,
                            ],
                            o_sb[:],
                        )

    _split_sync_waits(nc)
    return nc


def _get_nc():
    if "nc" not in _CACHE:
        _CACHE["nc"] = _build_nc()
    return _CACHE["nc"]


def kernel(hidden_states, W, b, coeff, mask, _trace=False, _trace_kwargs=None):
    import ml_dtypes

    bf16 = ml_dtypes.bfloat16
    fp8 = ml_dtypes.float8_e4m3

    nc = _get_nc()
    hs = np.asarray(hidden_states)
    Wf = np.asarray(W)
    bf = np.asarray(b, dtype=np.float32)
    cf = np.asarray(coeff, dtype=np.float32)
    mk = np.asarray(mask)

    in_maps = []
    for core in range(N_CORES):
        g, bj = core // BG, core % BG
        in_maps.append(
            {
                "h": np.ascontiguousarray(
                    hs[bj * B_SH : (bj + 1) * B_SH].transpose(0, 2, 1).astype(bf16)
                ),
                "W": np.ascontiguousarray(
                    Wf[g * O_SH : (g + 1) * O_SH].T.astype(bf16)
                ),
                "bias": np.ascontiguousarray(bf[g * O_SH : (g + 1) * O_SH]),
                "coeff": np.ascontiguousarray(cf[bj * B_SH : (bj + 1) * B_SH]),
                "mask": np.ascontiguousarray(
                    mk[bj * B_SH : (bj + 1) * B_SH, :, g * O_SH : (g + 1) * O_SH]
                    .astype(np.float32)
                    .astype(fp8)
                ),
            }
        )
    kwargs = {}
    if _trace:
        kwargs = {"trace": True, "trace_kwargs": _trace_kwargs or {}}
    # The first touch of the device after an abnormal process exit can fail
    # with NRT_EXEC_UNIT_UNRECOVERABLE; the failed attempt clears the wedged
    # state, so retry.
    last_err = None
    for attempt in range(3):
        try:
            res = run_bass_kernel_spmd(
                nc, in_maps, core_ids=list(range(N_CORES)), **kwargs
            )
            break
        except Exception as e:  # jax.errors.JaxRuntimeError etc.
            last_err = e
            try:
                import jax

                jax.clear_caches()
            except Exception:
                pass
            import time as _time

            _time.sleep(2.0)
    else:
        raise last_err
    _CACHE["last_results"] = res

    out = np.empty((B, S, O_FULL), dtype=np.float32)
    for core in range(N_CORES):
        g, bj = core // BG, core % BG
        out[bj * B_SH : (bj + 1) * B_SH, :, g * O_SH : (g + 1) * O_SH] = res.results[
            core
        ]["out"]
    return out


# revision 7
# speedup vs baseline: 1.0891x; 1.0891x over previous
"""Trainium2 Bass kernel for DiffCompressModule.

Reference computation (B=4, S=512, D_IN=D_OUT=4096):
    out = h @ W.T + b + coeff[b] * (h @ (2*mask[b] - 1))

Fused form used here (one matmul instead of two):
    out[b] = h[b] @ M_b + bias,   M_b = W.T + coeff[b] * (2*mask[b] - 1)

The host re-encodes each input tensor (layout + dtype only, no cross-tensor
math): h and W.T are shipped pre-transposed in bf16, the {0,1} mask as
fp8e4 (both values exact). On-device, ACT decodes t = 2c*mask - c (reading
fp8, writing bf16), DVE adds W.T to form M_b, and the fused matmul runs in
bf16 with fp32 PSUM accumulation. All DMAs are plain HWDGE copies; DMA cost
is charged on SBUF-side bytes, so per core it moves ~28MB-equivalent
(mask 8 + h 8 + W 8 + out 4) while PE does 512 x [128x128]@[128x512]
bf16 matmuls (~109us) - the kernel is PE-bound.

Sharding over 8 cores: 4 out-feature groups x 2 batch groups.
Each core: h [2,4096,512] bf16, W [4096,1024] bf16, bias [1024] f32,
coeff [2] f32, mask [2,4096,1024] fp8 -> out [2,512,1024] f32.
"""

import numpy as np

import concourse.bass as bass
import concourse.mybir as mybir
from concourse import tile
from concourse.bass_utils import run_bass_kernel_spmd

B, S, D = 4, 512, 4096
O_FULL = 4096
N_CORES = 8
OG, BG = 4, 2  # out-feature groups x batch groups
O_SH = O_FULL // OG  # 1024 out features per core
B_SH = B // BG  # 2 batches per core
HALF = 512  # PSUM bank holds 512 fp32 per partition
KC = D // 128  # 32 contraction chunks
SC = S // 128  # 4 s chunks
# kc-chunk grouping per (DMA + decode) unit: fine-grained at the start so
# the first matmul is ready early and the decode pipeline has runway,
# coarse after to amortize per-instruction overheads.
UNITS = (1, 1, 1, 1, 1, 1, 1, 1, 2, 2, 2, 2, 4, 4, 4, 4)
WARMUP_MM = 14  # PE warm-up matmuls on zeroed scratch (p-state ramp)
dt = mybir.dt

_CACHE = {}


def _split_sync_waits(nc, max_waits=1):
    # CoreV3 walrus rejects instructions with more than one semaphore wait
    # ("Too many sync wait commands"). Splitting the waits across preceding
    # same-engine NOPs is equivalent (the sequencer blocks on each in turn).
    ctr = 0
    for fn in nc.m.functions:
        for bb in fn.blocks:
            insts = bb.instructions
            if not any(
                i.sync_info is not None and len(i.sync_info.on_wait) > max_waits
                for i in insts
            ):
                continue
            new_list = []
            for ins in insts:
                si = ins.sync_info
                if si is not None and len(si.on_wait) > max_waits:
                    waits = list(si.on_wait)
                    head, tail = waits[:-max_waits], waits[-max_waits:]
                    for k in range(0, len(head), max_waits):
                        nop = mybir.InstNoOp(
                            name=f"waitsplit-{ctr}",
                            engine=ins.engine,
                            ins=[],
                            outs=[],
                            sync_info=mybir.SyncInfo(
                                on_wait=head[k : k + max_waits], on_update=[]
                            ),
                        )
                        ctr += 1
                        new_list.append(nop)
                    ins.sync_info = mybir.SyncInfo(
                        on_wait=tail, on_update=list(si.on_update)
                    )
                new_list.append(ins)
            bb.instructions = new_list


def _build_nc():
    nc = bass.Bass("TRN2", target_bir_lowering=False, debug=False)
    h = nc.dram_tensor("h", [B_SH, D, S], dt.bfloat16, kind="ExternalInput").ap()
    W = nc.dram_tensor("W", [D, O_SH], dt.bfloat16, kind="ExternalInput").ap()
    bias = nc.dram_tensor("bias", [O_SH], dt.float32, kind="ExternalInput").ap()
    coeff = nc.dram_tensor("coeff", [B_SH], dt.float32, kind="ExternalInput").ap()
    mask = nc.dram_tensor(
        "mask", [B_SH, D, O_SH], dt.float8e4, kind="ExternalInput"
    ).ap()
    out = nc.dram_tensor("out", [B_SH, S, O_SH], dt.float32, kind="ExternalOutput").ap()

    with tile.TileContext(nc) as tc:
        with (
            tc.tile_pool(name="const", bufs=1) as const_pool,
            # one W pool per unit size: pool slots are max-tile-size * bufs
            tc.tile_pool(
                name="wt1", bufs=sum(1 for u in UNITS if u == 1)
            ) as wt_pool1,
            tc.tile_pool(
                name="wt2", bufs=sum(1 for u in UNITS if u == 2)
            ) as wt_pool2,
            tc.tile_pool(
                name="wt4", bufs=sum(1 for u in UNITS if u == 4)
            ) as wt_pool4,
            tc.tile_pool(name="ht", bufs=6) as ht_pool,
            tc.tile_pool(name="mk", bufs=5) as mk_pool,
            tc.tile_pool(name="tt", bufs=2) as t_pool,
            tc.tile_pool(name="m", bufs=3) as m_pool,
            tc.tile_pool(name="ost", bufs=4) as out_pool,
            tc.tile_pool(name="acc", bufs=8, space="PSUM") as acc_pool,
        ):
            # coeff first: the decode chain of the very first unit needs c2.
            coeff_bc = const_pool.tile([128, B_SH], dt.float32)
            nc.sync.dma_start(
                coeff_bc[:], bass.AP(coeff.tensor, 0, [[0, 128], [1, B_SH]])
            )
            # PE warm-up: zeroed scratch matmuls keep the tensor engine busy
            # (ramping its p-state) while the first decode chain is in flight.
            wu_h = const_pool.tile([128, 128], dt.bfloat16)
            wu_m = const_pool.tile([128, HALF], dt.bfloat16)
            nc.vector.memset(wu_h[:], 0.0)
            nc.vector.memset(wu_m[:], 0.0)
            wu_acc = acc_pool.tile([128, HALF], dt.float32, tag="acc", name="wuacc")
            for i in range(WARMUP_MM):
                nc.tensor.matmul(
                    wu_acc[:],
                    wu_h[:],
                    wu_m[:],
                    start=(i == 0),
                    stop=(i == WARMUP_MM - 1),
                )
            c2 = const_pool.tile([128, B_SH], dt.float32)
            cneg = const_pool.tile([128, B_SH], dt.float32)
            nc.vector.tensor_scalar_mul(c2[:], coeff_bc[:], 2.0)
            nc.vector.tensor_scalar_mul(cneg[:], coeff_bc[:], -1.0)
            bias_bc = const_pool.tile([128, O_SH], dt.float32)

            wt = []  # per-unit W.T tiles [128, u*O_SH] bf16, resident

            for b in range(B_SH):
                accs = [
                    acc_pool.tile([128, HALF], dt.float32, tag="acc", name="acc")
                    for _ in range(SC * 2)
                ]
                kc = 0
                for ui, u in enumerate(UNITS):
                    kc0 = kc
                    if b == 0:
                        wt_pool = {1: wt_pool1, 2: wt_pool2, 4: wt_pool4}[u]
                        w_t = wt_pool.tile([128, u * O_SH], dt.bfloat16, name="wt")
                        nc.sync.dma_start(
                            w_t[:],
                            bass.AP(
                                W.tensor,
                                kc0 * 128 * O_SH,
                                [[O_SH, 128], [128 * O_SH, u], [1, O_SH]],
                            ),
                        )
                        wt.append(w_t)
                        if ui == 4:
                            # bias is only needed at the epilogue; issue its
                            # broadcast DMA off the startup critical path.
                            nc.sync.dma_start(
                                bias_bc[:],
                                bass.AP(bias.tensor, 0, [[0, 128], [1, O_SH]]),
                            )
                    mk = mk_pool.tile([128, u * O_SH], dt.float8e4, name="mk")
                    nc.sync.dma_start(
                        mk[:],
                        bass.AP(
                            mask.tensor,
                            (b * D + kc0 * 128) * O_SH,
                            [[O_SH, 128], [128 * O_SH, u], [1, O_SH]],
                        ),
                    )
                    ht = ht_pool.tile([128, u * S], dt.bfloat16, name="ht")
                    nc.scalar.dma_start(
                        ht[:],
                        bass.AP(
                            h.tensor,
                            (b * D + kc0 * 128) * S,
                            [[S, 128], [128 * S, u], [1, S]],
                        ),
                    )
                    t_sb = t_pool.tile([128, u * O_SH], dt.bfloat16, name="tsb")
                    nc.scalar.activation(
                        t_sb[:],
                        mk[:],
                        mybir.ActivationFunctionType.Identity,
                        bias=cneg[:, b : b + 1],
                        scale=c2[:, b : b + 1],
                    )
                    m = m_pool.tile([128, u * O_SH], dt.bfloat16, name="m")
                    nc.vector.tensor_tensor(
                        m[:], t_sb[:], wt[ui][:], mybir.AluOpType.add
                    )
                    for j in range(u):
                        for sc in range(SC):
                            for half in range(2):
                                nc.tensor.matmul(
                                    accs[sc * 2 + half][:],
                                    ht[:, j * S + sc * 128 : j * S + (sc + 1) * 128],
                                    m[
                                        :,
                                        j * O_SH + half * HALF : j * O_SH
                                        + (half + 1) * HALF,
                                    ],
                                    start=(kc == 0),
                                    stop=(kc == KC - 1),
                                )
                        kc += 1
                for sc in range(SC):
                    for half in range(2):
                        o_sb = out_pool.tile([128, HALF], dt.float32, name="osb")
                        nc.vector.tensor_tensor(
                            o_sb[:],
                            accs[sc * 2 + half][:],
                            bias_bc[:, half * HALF : (half + 1) * HALF],
                            mybir.AluOpType.add,
                        )
                        nc.gpsimd.dma_start(
                            out[
                                b,
                                sc * 128 : (sc + 1) * 128,
                                half * HALF : (half + 1) * HALF,
                            ],
                            o_sb[:],
                        )

    _split_sync_waits(nc)
    return nc


def _get_nc():
    if "nc" not in _CACHE:
        _CACHE["nc"] = _build_nc()
    return _CACHE["nc"]


def kernel(hidden_states, W, b, coeff, mask, _trace=False, _trace_kwargs=None):
    import ml_dtypes

    bf16 = ml_dtypes.bfloat16
    fp8 = ml_dtypes.float8_e4m3

    nc = _get_nc()
    hs = np.asarray(hidden_states)
    Wf = np.asarray(W)
    bf = np.asarray(b, dtype=np.float32)
    cf = np.asarray(coeff, dtype=np.float32)
    mk = np.asarray(mask)

    in_maps = []
    for core in range(N_CORES):
        g, bj = core // BG, core % BG
        in_maps.append(
            {
                "h": np.ascontiguousarray(
                    hs[bj * B_SH : (bj + 1) * B_SH].transpose(0, 2, 1).astype(bf16)
                ),
                "W": np.ascontiguousarray(
                    Wf[g * O_SH : (g + 1) * O_SH].T.astype(bf16)
                ),
                "bias": np.ascontiguousarray(bf[g * O_SH : (g + 1) * O_SH]),
                "coeff": np.ascontiguousarray(cf[bj * B_SH : (bj + 1) * B_SH]),
                "mask": np.ascontiguousarray(
                    mk[bj * B_SH : (bj + 1) * B_SH, :, g * O_SH : (g + 1) * O_SH]
                    .astype(np.float32)
                    .astype(fp8)
                ),
            }
        )
    kwargs = {}
    if _trace:
        kwargs = {"trace": True, "trace_kwargs": _trace_kwargs or {}}
    # The first touch of the device after an abnormal process exit can fail
    # with NRT_EXEC_UNIT_UNRECOVERABLE; the failed attempt clears the wedged
    # state, so retry.
    last_err = None
    for attempt in range(3):
        try:
            res = run_bass_kernel_spmd(
                nc, in_maps, core_ids=list(range(N_CORES)), **kwargs
            )
            break
        except Exception as e:  # jax.errors.JaxRuntimeError etc.
            last_err = e
            try:
                import jax

                jax.clear_caches()
            except Exception:
                pass
            import time as _time

            _time.sleep(2.0)
    else:
        raise last_err
    _CACHE["last_results"] = res

    out = np.empty((B, S, O_FULL), dtype=np.float32)
    for core in range(N_CORES):
        g, bj = core // BG, core % BG
        out[bj * B_SH : (bj + 1) * B_SH, :, g * O_SH : (g + 1) * O_SH] = res.results[
            core
        ]["out"]
    return out


# revision 9
# speedup vs baseline: 1.1160x; 1.0247x over previous
"""Trainium2 Bass kernel for DiffCompressModule.

Reference computation (B=4, S=512, D_IN=D_OUT=4096):
    out = h @ W.T + b + coeff[b] * (h @ (2*mask[b] - 1))

Fused form used here (one matmul instead of two):
    out[b] = h[b] @ M_b + bias,   M_b = W.T + coeff[b] * (2*mask[b] - 1)

The host re-encodes each input tensor (layout + dtype only, no cross-tensor
math): h and W.T are shipped pre-transposed in bf16, the {0,1} mask as
fp8e4 (both values exact). On-device, ACT decodes t = 2c*mask - c (reading
fp8, writing bf16), DVE adds W.T to form M_b, and the fused matmul runs in
bf16 with fp32 PSUM accumulation. All DMAs are plain HWDGE copies; DMA cost
is charged on SBUF-side bytes, so per core it moves ~28MB-equivalent
(mask 8 + h 8 + W 8 + out 4) while PE does 512 x [128x128]@[128x512]
bf16 matmuls (~109us) - the kernel is PE-bound.

Sharding over 8 cores: 4 out-feature groups x 2 batch groups.
Each core: h [2,4096,512] bf16, W [4096,1024] bf16, bias [1024] f32,
coeff [2] f32, mask [2,4096,1024] fp8 -> out [2,512,1024] f32.
"""

import numpy as np

import concourse.bass as bass
import concourse.mybir as mybir
from concourse import tile
from concourse.bass_utils import run_bass_kernel_spmd

B, S, D = 4, 512, 4096
O_FULL = 4096
N_CORES = 8
OG, BG = 4, 2  # out-feature groups x batch groups
O_SH = O_FULL // OG  # 1024 out features per core
B_SH = B // BG  # 2 batches per core
HALF = 512  # PSUM bank holds 512 fp32 per partition
KC = D // 128  # 32 contraction chunks
SC = S // 128  # 4 s chunks
# kc-chunk grouping per (DMA + decode) unit: fine-grained at the start so
# the first matmul is ready early and the decode pipeline has runway,
# coarse after to amortize per-instruction overheads.
UNITS = (1, 1, 1, 1, 1, 1, 1, 1, 2, 2, 2, 2, 4, 4, 4, 4)
WARMUP_MM = 14  # PE warm-up matmuls on zeroed scratch (p-state ramp)
dt = mybir.dt

_CACHE = {}


def _split_sync_waits(nc, max_waits=1):
    # CoreV3 walrus rejects instructions with more than one semaphore wait
    # ("Too many sync wait commands"). Splitting the waits across preceding
    # same-engine NOPs is equivalent (the sequencer blocks on each in turn).
    ctr = 0
    for fn in nc.m.functions:
        for bb in fn.blocks:
            insts = bb.instructions
            if not any(
                i.sync_info is not None and len(i.sync_info.on_wait) > max_waits
                for i in insts
            ):
                continue
            new_list = []
            for ins in insts:
                si = ins.sync_info
                if si is not None and len(si.on_wait) > max_waits:
                    waits = list(si.on_wait)
                    head, tail = waits[:-max_waits], waits[-max_waits:]
                    for k in range(0, len(head), max_waits):
                        nop = mybir.InstNoOp(
                            name=f"waitsplit-{ctr}",
                            engine=ins.engine,
                            ins=[],
                            outs=[],
                            sync_info=mybir.SyncInfo(
                                on_wait=head[k : k + max_waits], on_update=[]
                            ),
                        )
                        ctr += 1
                        new_list.append(nop)
                    ins.sync_info = mybir.SyncInfo(
                        on_wait=tail, on_update=list(si.on_update)
                    )
                new_list.append(ins)
            bb.instructions = new_list


def _build_nc():
    nc = bass.Bass("TRN2", target_bir_lowering=False, debug=False)
    h = nc.dram_tensor("h", [B_SH, D, S], dt.bfloat16, kind="ExternalInput").ap()
    W = nc.dram_tensor("W", [D, O_SH], dt.bfloat16, kind="ExternalInput").ap()
    bias = nc.dram_tensor("bias", [O_SH], dt.float32, kind="ExternalInput").ap()
    coeff = nc.dram_tensor("coeff", [B_SH], dt.float32, kind="ExternalInput").ap()
    mask = nc.dram_tensor(
        "mask", [B_SH, D, O_SH], dt.float8e4, kind="ExternalInput"
    ).ap()
    out = nc.dram_tensor("out", [B_SH, S, O_SH], dt.float32, kind="ExternalOutput").ap()

    with tile.TileContext(nc) as tc:
        with (
            tc.tile_pool(name="const", bufs=1) as const_pool,
            # one W pool per unit size: pool slots are max-tile-size * bufs
            tc.tile_pool(
                name="wt1", bufs=sum(1 for u in UNITS if u == 1)
            ) as wt_pool1,
            tc.tile_pool(
                name="wt2", bufs=sum(1 for u in UNITS if u == 2)
            ) as wt_pool2,
            tc.tile_pool(
                name="wt4", bufs=sum(1 for u in UNITS if u == 4)
            ) as wt_pool4,
            tc.tile_pool(name="ht", bufs=4) as ht_pool,
            tc.tile_pool(name="mk", bufs=6) as mk_pool,
            tc.tile_pool(name="tt", bufs=3) as t_pool,
            tc.tile_pool(name="m", bufs=4) as m_pool,
            tc.tile_pool(name="ost", bufs=4) as out_pool,
            tc.tile_pool(name="acc", bufs=8, space="PSUM") as acc_pool,
        ):
            # coeff first: the decode chain of the very first unit needs c2.
            coeff_bc = const_pool.tile([128, B_SH], dt.float32)
            nc.sync.dma_start(
                coeff_bc[:], bass.AP(coeff.tensor, 0, [[0, 128], [1, B_SH]])
            )
            # PE warm-up: zeroed scratch matmuls keep the tensor engine busy
            # (ramping its p-state) while the first decode chain is in flight.
            wu_h = const_pool.tile([128, 128], dt.bfloat16)
            wu_m = const_pool.tile([128, HALF], dt.bfloat16)
            nc.vector.memset(wu_h[:], 0.0)
            nc.vector.memset(wu_m[:], 0.0)
            wu_acc = acc_pool.tile([128, HALF], dt.float32, tag="acc", name="wuacc")
            for i in range(WARMUP_MM):
                nc.tensor.matmul(
                    wu_acc[:],
                    wu_h[:],
                    wu_m[:],
                    start=(i == 0),
                    stop=(i == WARMUP_MM - 1),
                )
            c2 = const_pool.tile([128, B_SH], dt.float32)
            cneg = const_pool.tile([128, B_SH], dt.float32)
            nc.vector.tensor_scalar_mul(c2[:], coeff_bc[:], 2.0)
            nc.vector.tensor_scalar_mul(cneg[:], coeff_bc[:], -1.0)
            bias_bc = const_pool.tile([128, O_SH], dt.float32)

            wt = []  # per-unit W.T tiles [128, u*O_SH] bf16, resident

            for b in range(B_SH):
                accs = [
                    acc_pool.tile([128, HALF], dt.float32, tag="acc", name="acc")
                    for _ in range(SC * 2)
                ]
                kc = 0
                for ui, u in enumerate(UNITS):
                    kc0 = kc
                    mk = mk_pool.tile([128, u * O_SH], dt.float8e4, name="mk")
                    nc.sync.dma_start(
                        mk[:],
                        bass.AP(
                            mask.tensor,
                            (b * D + kc0 * 128) * O_SH,
                            [[O_SH, 128], [128 * O_SH, u], [1, O_SH]],
                        ),
                    )
                    if b == 0:
                        wt_pool = {1: wt_pool1, 2: wt_pool2, 4: wt_pool4}[u]
                        w_t = wt_pool.tile([128, u * O_SH], dt.bfloat16, name="wt")
                        nc.sync.dma_start(
                            w_t[:],
                            bass.AP(
                                W.tensor,
                                kc0 * 128 * O_SH,
                                [[O_SH, 128], [128 * O_SH, u], [1, O_SH]],
                            ),
                        )
                        wt.append(w_t)
                        if ui == 4:
                            # bias is only needed at the epilogue; issue its
                            # broadcast DMA off the startup critical path.
                            nc.sync.dma_start(
                                bias_bc[:],
                                bass.AP(bias.tensor, 0, [[0, 128], [1, O_SH]]),
                            )
                    ht = ht_pool.tile([128, u * S], dt.bfloat16, name="ht")
                    nc.scalar.dma_start(
                        ht[:],
                        bass.AP(
                            h.tensor,
                            (b * D + kc0 * 128) * S,
                            [[S, 128], [128 * S, u], [1, S]],
                        ),
                    )
                    t_sb = t_pool.tile([128, u * O_SH], dt.bfloat16, name="tsb")
                    nc.scalar.activation(
                        t_sb[:],
                        mk[:],
                        mybir.ActivationFunctionType.Identity,
                        bias=cneg[:, b : b + 1],
                        scale=c2[:, b : b + 1],
                    )
                    m = m_pool.tile([128, u * O_SH], dt.bfloat16, name="m")
                    nc.vector.tensor_tensor(
                        m[:], t_sb[:], wt[ui][:], mybir.AluOpType.add
                    )
                    for j in range(u):
                        for sc in range(SC):
                            for half in range(2):
                                nc.tensor.matmul(
                                    accs[sc * 2 + half][:],
                                    ht[:, j * S + sc * 128 : j * S + (sc + 1) * 128],
                                    m[
                                        :,
                                        j * O_SH + half * HALF : j * O_SH
                                        + (half + 1) * HALF,
                                    ],
                                    start=(kc == 0),
                                    stop=(kc == KC - 1),
                                )
                        kc += 1
                for sc in range(SC):
                    for half in range(2):
                        o_sb = out_pool.tile([128, HALF], dt.float32, name="osb")
                        nc.vector.tensor_tensor(
                            o_sb[:],
                            accs[sc * 2 + half][:],
                            bias_bc[:, half * HALF : (half + 1) * HALF],
                            mybir.AluOpType.add,
                        )
                        nc.gpsimd.dma_start(
                            out[
                                b,
                                sc * 128 : (sc + 1) * 128,
                                half * HALF : (half + 1) * HALF,
                            ],
                            o_sb[:],
                        )

    _split_sync_waits(nc)
    return nc


def _get_nc():
    if "nc" not in _CACHE:
        _CACHE["nc"] = _build_nc()
    return _CACHE["nc"]


def kernel(hidden_states, W, b, coeff, mask, _trace=False, _trace_kwargs=None):
    import ml_dtypes

    bf16 = ml_dtypes.bfloat16
    fp8 = ml_dtypes.float8_e4m3

    nc = _get_nc()
    hs = np.asarray(hidden_states)
    Wf = np.asarray(W)
    bf = np.asarray(b, dtype=np.float32)
    cf = np.asarray(coeff, dtype=np.float32)
    mk = np.asarray(mask)

    in_maps = []
    for core in range(N_CORES):
        g, bj = core // BG, core % BG
        in_maps.append(
            {
                "h": np.ascontiguousarray(
                    hs[bj * B_SH : (bj + 1) * B_SH].transpose(0, 2, 1).astype(bf16)
                ),
                "W": np.ascontiguousarray(
                    Wf[g * O_SH : (g + 1) * O_SH].T.astype(bf16)
                ),
                "bias": np.ascontiguousarray(bf[g * O_SH : (g + 1) * O_SH]),
                "coeff": np.ascontiguousarray(cf[bj * B_SH : (bj + 1) * B_SH]),
                "mask": np.ascontiguousarray(
                    mk[bj * B_SH : (bj + 1) * B_SH, :, g * O_SH : (g + 1) * O_SH]
                    .astype(np.float32)
                    .astype(fp8)
                ),
            }
        )
    kwargs = {}
    if _trace:
        kwargs = {"trace": True, "trace_kwargs": _trace_kwargs or {}}
    # The first touch of the device after an abnormal process exit can fail
    # with NRT_EXEC_UNIT_UNRECOVERABLE; the failed attempt clears the wedged
    # state, so retry.
    last_err = None
    for attempt in range(3):
        try:
            res = run_bass_kernel_spmd(
                nc, in_maps, core_ids=list(range(N_CORES)), **kwargs
            )
            break
        except Exception as e:  # jax.errors.JaxRuntimeError etc.
            last_err = e
            try:
                import jax

                jax.clear_caches()
            except Exception:
                pass
            import time as _time

            _time.sleep(2.0)
    else:
        raise last_err
    _CACHE["last_results"] = res

    out = np.empty((B, S, O_FULL), dtype=np.float32)
    for core in range(N_CORES):
        g, bj = core // BG, core % BG
        out[bj * B_SH : (bj + 1) * B_SH, :, g * O_SH : (g + 1) * O_SH] = res.results[
            core
        ]["out"]
    return out


# revision 11
# speedup vs baseline: 1.1696x; 1.0480x over previous
"""Trainium2 Bass kernel for DiffCompressModule.

Reference computation (B=4, S=512, D_IN=D_OUT=4096):
    out = h @ W.T + b + coeff[b] * (h @ (2*mask[b] - 1))

Fused form used here (one matmul instead of two):
    out[b] = h[b] @ M_b + bias,   M_b = W.T + coeff[b] * (2*mask[b] - 1)

The host re-encodes each input tensor (layout + dtype only, no cross-tensor
math): h and W.T are shipped pre-transposed in bf16, the {0,1} mask as
fp8e4 (both values exact). On-device, ACT decodes t = 2c*mask - c (reading
fp8, writing bf16), DVE adds W.T to form M_b, and the fused matmul runs in
bf16 with fp32 PSUM accumulation. All DMAs are plain HWDGE copies; DMA cost
is charged on SBUF-side bytes, so per core it moves ~28MB-equivalent
(mask 8 + h 8 + W 8 + out 4) while PE does 512 x [128x128]@[128x512]
bf16 matmuls (~109us) - the kernel is PE-bound.

Sharding over 8 cores: 4 out-feature groups x 2 batch groups.
Each core: h [2,4096,512] bf16, W [4096,1024] bf16, bias [1024] f32,
coeff [2] f32, mask [2,4096,1024] fp8 -> out [2,512,1024] f32.
"""

import numpy as np

import concourse.bass as bass
import concourse.mybir as mybir
from concourse import tile
from concourse.bass_utils import run_bass_kernel_spmd

B, S, D = 4, 512, 4096
O_FULL = 4096
N_CORES = 8
OG, BG = 4, 2  # out-feature groups x batch groups
O_SH = O_FULL // OG  # 1024 out features per core
B_SH = B // BG  # 2 batches per core
HALF = 512  # PSUM bank holds 512 fp32 per partition
KC = D // 128  # 32 contraction chunks
SC = S // 128  # 4 s chunks
# kc-chunk grouping per (DMA + decode) unit: fine-grained at the start so
# the first matmul is ready early and the decode pipeline has runway,
# coarse after to amortize per-instruction overheads.
UNITS = (1, 1, 1, 1, 1, 1, 1, 1, 2, 2, 2, 2, 4, 4, 4, 4)
WARMUP_MM = 14  # PE warm-up matmuls on zeroed scratch (p-state ramp)
dt = mybir.dt

_CACHE = {}


def _split_sync_waits(nc, max_waits=1):
    # CoreV3 walrus rejects instructions with more than one semaphore wait
    # ("Too many sync wait commands"). Splitting the waits across preceding
    # same-engine NOPs is equivalent (the sequencer blocks on each in turn).
    ctr = 0
    for fn in nc.m.functions:
        for bb in fn.blocks:
            insts = bb.instructions
            if not any(
                i.sync_info is not None and len(i.sync_info.on_wait) > max_waits
                for i in insts
            ):
                continue
            new_list = []
            for ins in insts:
                si = ins.sync_info
                if si is not None and len(si.on_wait) > max_waits:
                    waits = list(si.on_wait)
                    head, tail = waits[:-max_waits], waits[-max_waits:]
                    for k in range(0, len(head), max_waits):
                        nop = mybir.InstNoOp(
                            name=f"waitsplit-{ctr}",
                            engine=ins.engine,
                            ins=[],
                            outs=[],
                            sync_info=mybir.SyncInfo(
                                on_wait=head[k : k + max_waits], on_update=[]
                            ),
                        )
                        ctr += 1
                        new_list.append(nop)
                    ins.sync_info = mybir.SyncInfo(
                        on_wait=tail, on_update=list(si.on_update)
                    )
                new_list.append(ins)
            bb.instructions = new_list


def _build_nc():
    nc = bass.Bass("TRN2", target_bir_lowering=False, debug=False)
    h = nc.dram_tensor("h", [B_SH, D, S], dt.bfloat16, kind="ExternalInput").ap()
    W = nc.dram_tensor("W", [D, O_SH], dt.bfloat16, kind="ExternalInput").ap()
    bias = nc.dram_tensor("bias", [O_SH], dt.float32, kind="ExternalInput").ap()
    coeff = nc.dram_tensor("coeff", [B_SH], dt.float32, kind="ExternalInput").ap()
    mask = nc.dram_tensor(
        "mask", [B_SH, D, O_SH], dt.float8e4, kind="ExternalInput"
    ).ap()
    out = nc.dram_tensor("out", [B_SH, S, O_SH], dt.float32, kind="ExternalOutput").ap()

    with tile.TileContext(nc) as tc:
        with (
            tc.tile_pool(name="const", bufs=1) as const_pool,
            # one W pool per unit size: pool slots are max-tile-size * bufs
            tc.tile_pool(
                name="wt1", bufs=sum(1 for u in UNITS if u == 1)
            ) as wt_pool1,
            tc.tile_pool(
                name="wt2", bufs=sum(1 for u in UNITS if u == 2)
            ) as wt_pool2,
            tc.tile_pool(
                name="wt4", bufs=sum(1 for u in UNITS if u == 4)
            ) as wt_pool4,
            tc.tile_pool(name="ht", bufs=4) as ht_pool,
            tc.tile_pool(name="mk", bufs=6) as mk_pool,
            tc.tile_pool(name="tt", bufs=3) as t_pool,
            tc.tile_pool(name="m", bufs=4) as m_pool,
            tc.tile_pool(name="ost", bufs=4) as out_pool,
            tc.tile_pool(name="acc", bufs=8, space="PSUM") as acc_pool,
        ):
            # coeff first: the decode chain of the very first unit needs c2.
            coeff_bc = const_pool.tile([128, B_SH], dt.float32)
            nc.sync.dma_start(
                coeff_bc[:], bass.AP(coeff.tensor, 0, [[0, 128], [1, B_SH]])
            )
            # PE warm-up: zeroed scratch matmuls keep the tensor engine busy
            # (ramping its p-state) while the first decode chain is in flight.
            wu_h = const_pool.tile([128, 128], dt.bfloat16)
            wu_m = const_pool.tile([128, HALF], dt.bfloat16)
            nc.vector.memset(wu_h[:], 0.0)
            nc.vector.memset(wu_m[:], 0.0)
            wu_acc = acc_pool.tile([128, HALF], dt.float32, tag="acc", name="wuacc")
            for i in range(WARMUP_MM):
                nc.tensor.matmul(
                    wu_acc[:],
                    wu_h[:],
                    wu_m[:],
                    start=(i == 0),
                    stop=(i == WARMUP_MM - 1),
                )
            c2 = const_pool.tile([128, B_SH], dt.float32)
            cneg = const_pool.tile([128, B_SH], dt.float32)
            nc.vector.tensor_scalar_mul(c2[:], coeff_bc[:], 2.0)
            nc.vector.tensor_scalar_mul(cneg[:], coeff_bc[:], -1.0)
            bias_bc = const_pool.tile([128, O_SH], dt.float32)

            wt = []  # per-unit W.T tiles [128, u*O_SH] bf16, resident

            def unit_chain(b, ui, u, kc0):
                """DMAs + decode (ACT) + W-add (DVE) for one kc unit."""
                mk = mk_pool.tile([128, u * O_SH], dt.float8e4, name="mk")
                nc.sync.dma_start(
                    mk[:],
                    bass.AP(
                        mask.tensor,
                        (b * D + kc0 * 128) * O_SH,
                        [[O_SH, 128], [128 * O_SH, u], [1, O_SH]],
                    ),
                )
                if b == 0:
                    wt_pool = {1: wt_pool1, 2: wt_pool2, 4: wt_pool4}[u]
                    w_t = wt_pool.tile([128, u * O_SH], dt.bfloat16, name="wt")
                    nc.sync.dma_start(
                        w_t[:],
                        bass.AP(
                            W.tensor,
                            kc0 * 128 * O_SH,
                            [[O_SH, 128], [128 * O_SH, u], [1, O_SH]],
                        ),
                    )
                    wt.append(w_t)
                    if ui == 4:
                        # bias is only needed at the epilogue; issue its
                        # broadcast DMA off the startup critical path.
                        nc.sync.dma_start(
                            bias_bc[:],
                            bass.AP(bias.tensor, 0, [[0, 128], [1, O_SH]]),
                        )
                ht = ht_pool.tile([128, u * S], dt.bfloat16, name="ht")
                nc.scalar.dma_start(
                    ht[:],
                    bass.AP(
                        h.tensor,
                        (b * D + kc0 * 128) * S,
                        [[S, 128], [128 * S, u], [1, S]],
                    ),
                )
                t_sb = t_pool.tile([128, u * O_SH], dt.bfloat16, name="tsb")
                nc.scalar.activation(
                    t_sb[:],
                    mk[:],
                    mybir.ActivationFunctionType.Identity,
                    bias=cneg[:, b : b + 1],
                    scale=c2[:, b : b + 1],
                )
                m = m_pool.tile([128, u * O_SH], dt.bfloat16, name="m")
                nc.vector.tensor_tensor(m[:], t_sb[:], wt[ui][:], mybir.AluOpType.add)
                return ht, m

            def acc_block(accs, i, items):
                """All matmuls for accumulator i over the given units."""
                sc, half = i // 2, i % 2
                for u, ht, m, kc0 in items:
                    for j in range(u):
                        nc.tensor.matmul(
                            accs[i][:],
                            ht[:, j * S + sc * 128 : j * S + (sc + 1) * 128],
                            m[
                                :,
                                j * O_SH + half * HALF : j * O_SH
                                + (half + 1) * HALF,
                            ],
                            start=(kc0 + j == 0),
                            stop=(kc0 + j == KC - 1),
                        )

            def epilogue(b, accs, i):
                sc, half = i // 2, i % 2
                o_sb = out_pool.tile([128, HALF], dt.float32, name="osb")
                nc.vector.tensor_tensor(
                    o_sb[:],
                    accs[i][:],
                    bias_bc[:, half * HALF : (half + 1) * HALF],
                    mybir.AluOpType.add,
                )
                nc.gpsimd.dma_start(
                    out[
                        b,
                        sc * 128 : (sc + 1) * 128,
                        half * HALF : (half + 1) * HALF,
                    ],
                    o_sb[:],
                )

            NU = len(UNITS)
            pending = None  # (b, accs) whose epilogues are deferred
            for b in range(B_SH):
                accs = [
                    acc_pool.tile([128, HALF], dt.float32, tag="acc", name="acc")
                    for _ in range(SC * 2)
                ]
                kc = 0
                tail = []  # last two units, matmul'd acc-major as one stretch
                for ui, u in enumerate(UNITS):
                    ht_m = unit_chain(b, ui, u, kc)
                    if ui >= NU - 2:
                        tail.append((u, *ht_m, kc))
                    else:
                        acc_items = [(u, *ht_m, kc)]
                        for i in range(SC * 2):
                            acc_block(accs, i, acc_items)
                    if pending is not None and ui == 2:
                        # previous batch's epilogues: by now this batch's
                        # first three m-tiles are built, and the previous
                        # batch's accumulators (acc-major tail) have all
                        # stopped well before its matmul stream ends.
                        pb, paccs = pending
                        for i in range(SC * 2):
                            epilogue(pb, paccs, i)
                        pending = None
                    kc += u
                # fused acc-major tail over the last two units: each
                # accumulator finishes all its remaining contraction chunks
                # early, opening a long window for epilogues + PSUM reuse.
                for i in range(SC * 2):
                    acc_block(accs, i, tail)
                    if b == B_SH - 1:
                        epilogue(b, accs, i)
                if b < B_SH - 1:
                    pending = (b, accs)

    _split_sync_waits(nc)
    return nc


def _get_nc():
    if "nc" not in _CACHE:
        _CACHE["nc"] = _build_nc()
    return _CACHE["nc"]


def kernel(hidden_states, W, b, coeff, mask, _trace=False, _trace_kwargs=None):
    import ml_dtypes

    bf16 = ml_dtypes.bfloat16
    fp8 = ml_dtypes.float8_e4m3

    nc = _get_nc()
    hs = np.asarray(hidden_states)
    Wf = np.asarray(W)
    bf = np.asarray(b, dtype=np.float32)
    cf = np.asarray(coeff, dtype=np.float32)
    mk = np.asarray(mask)

    in_maps = []
    for core in range(N_CORES):
        g, bj = core // BG, core % BG
        in_maps.append(
            {
                "h": np.ascontiguousarray(
                    hs[bj * B_SH : (bj + 1) * B_SH].transpose(0, 2, 1).astype(bf16)
                ),
                "W": np.ascontiguousarray(
                    Wf[g * O_SH : (g + 1) * O_SH].T.astype(bf16)
                ),
                "bias": np.ascontiguousarray(bf[g * O_SH : (g + 1) * O_SH]),
                "coeff": np.ascontiguousarray(cf[bj * B_SH : (bj + 1) * B_SH]),
                "mask": np.ascontiguousarray(
                    mk[bj * B_SH : (bj + 1) * B_SH, :, g * O_SH : (g + 1) * O_SH]
                    .astype(np.float32)
                    .astype(fp8)
                ),
            }
        )
    kwargs = {}
    if _trace:
        kwargs = {"trace": True, "trace_kwargs": _trace_kwargs or {}}
    # The first touch of the device after an abnormal process exit can fail
    # with NRT_EXEC_UNIT_UNRECOVERABLE; the failed attempt clears the wedged
    # state, so retry.
    last_err = None
    for attempt in range(3):
        try:
            res = run_bass_kernel_spmd(
                nc, in_maps, core_ids=list(range(N_CORES)), **kwargs
            )
            break
        except Exception as e:  # jax.errors.JaxRuntimeError etc.
            last_err = e
            try:
                import jax

                jax.clear_caches()
            except Exception:
                pass
            import time as _time

            _time.sleep(2.0)
    else:
        raise last_err
    _CACHE["last_results"] = res

    out = np.empty((B, S, O_FULL), dtype=np.float32)
    for core in range(N_CORES):
        g, bj = core // BG, core % BG
        out[bj * B_SH : (bj + 1) * B_SH, :, g * O_SH : (g + 1) * O_SH] = res.results[
            core
        ]["out"]
    return out
